# revision 17
# baseline (speedup 1.0000x reference)
"""Trainium2 Bass kernel for nn_AdvancedSpikingChatModel.

Model: spike-encode embeddings -> 6 spiking-transformer blocks (LIF gates +
decaying linear-attention recurrence over T=16) -> LIF output head with
spike-count accumulation over V=32000 vocab.

Strategy (8 NeuronCores, SPMD, two launches):
  Launch 1 (blocks): data-parallel over the 256 folded (b,s) rows, 32/core.
    Features on partitions, (t, row) on the free dim; weights stationary.
  Launch 2 (head): vocab-parallel, 4096 padded cols/core, all 256 rows.

Precision: matmuls run as fp16 hi/lo split passes (x@W = xh@Wh + xl@Wh +
xh@Wl accumulated in fp32 PSUM; dropped xl@Wl term ~2^-22) — fp32-grade
results at the PE's fp16 rate (fp32 matmuls cost ~2.6x on TRN2). The LIF
threshold compare (v >= 1) makes anything coarser (bf16/fp32r) flip spikes.
Spike matrices (0/1) are exact in fp16, so spike-side matmuls use 2 passes.

LIF decay 0.5 folded into weights: w' = 0.5*(min(w,1) - (w>=1)) + a, emitted
as ONE custom DVE op per step; spikes s = (w >= 1) recovered in one batched
GPSIMD pass per scan; spike counts via add-tree (GPSIMD + DVE).
"""

import numpy as np

import concourse.mybir as mybir
import concourse.tile as tile
from concourse import bacc
from concourse.bass_utils import run_bass_kernel_spmd

F32 = mybir.dt.float32
F16 = mybir.dt.float16
OP = mybir.AluOpType
AF = mybir.ActivationFunctionType

B, S, D, T, L, F, V = 2, 128, 256, 16, 6, 1024, 32000
N = B * S
NCORE = 8
R = N // NCORE       # 32 rows/core in launch 1
TR = T * R           # 512
KC = D // 128
FC = F // 128
VPAD = 32768
VSH = VPAD // NCORE  # 4096
VCH = VSH // 128     # 32 chunks
TN = T * N           # 4096
EPS = 1e-5

# fp16 weight slab offsets (fp16 words per partition, per layer)
GH_OFF = 0
GL_OFF = GH_OFF + 12 * 128
WOH_OFF = GL_OFF + 12 * 128
WOL_OFF = WOH_OFF + 4 * 128
W1H_OFF = WOL_OFF + 4 * 128
W1L_OFF = W1H_OFF + 16 * 128
W2H_OFF = W1L_OFF + 16 * 128
W2L_OFF = W2H_OFF + 16 * 128
W16 = W2L_OFF + 16 * 128
# fp32 smalls: b1(8) b2(2) ln(8)
B1_OFF = 0
B2_OFF = 8
LN_OFF = 10
WS = 18

_LIF_OP = None


def _get_lif_op():
    """Register the fused LIF step as a local custom DVE op:
    out = (min(w,1) - (w>=1))*0.5 + a."""
    global _LIF_OP
    if _LIF_OP is not None:
        return _LIF_OP
    from concourse.dve_spec import Spec, Src0, Src1, C0, One, minn, lower
    from concourse.dve_ops import (
        DveOp, OPS, _SUB_OPCODE_FOR_NAME, CUSTOM_DVE_SPECS)
    from concourse.dve_uop import DveOpSpec

    name = "LIF_STEP_ANT"
    if name not in _SUB_OPCODE_FOR_NAME:
        body = (minn(Src0, One) - (Src0 >= One)) * C0 + Src1
        spec = Spec(
            body=body,
            reference=lambda in0, in1, s0, s1, imm2:
                (np.minimum(in0, 1.0) - (in0 >= 1.0)) * s0 + in1,
        )
        op = DveOp(name, spec, subdim=False, uops_sha={})
        row = 1 + len(OPS)
        OPS.append(op)
        _SUB_OPCODE_FOR_NAME[name] = row
        CUSTOM_DVE_SPECS[name] = spec
        for ver in ("v3",):
            s = DveOpSpec(name=name, opcode=row, uops=lower(spec, ver=ver),
                          rd1_en=True)
            op.uops_sha[ver] = s.sha(ver)
        _LIF_OP = op
    else:
        _LIF_OP = next(o for o in OPS if o.name == name)
    return _LIF_OP


def _sigmoid(x):
    return 1.0 / (1.0 + np.exp(-x))


def _encode_spikes(input_ids, token_embedding, pos_embedding, noise, unif):
    """Host-side rate coding; (0.7*rate + 0.3*temp > 0.5) == rate exactly."""
    emb = token_embedding[input_ids] + pos_embedding[None, :S]
    p = np.clip(_sigmoid(emb) * 0.8 + 0.1 + noise * 0.05, 0.0, 1.0)
    return (unif < p[None]).astype(np.float32)


def _split16(x):
    hi = x.astype(np.float16)
    lo = (x - hi.astype(np.float32)).astype(np.float16)
    return hi, lo


def _mm16(nc, ps, passes, dst_ap, bias=0.0, name="mmb", free=512):
    """Accumulate fp16 matmul passes into one PSUM bank, ACT-copy(+bias) out."""
    bank = ps.tile([128, free], F32, tag="mm", name=name, bufs=4)
    npass = len(passes)
    for i, (lhsT, rhs) in enumerate(passes):
        nc.tensor.matmul(bank[:], lhsT, rhs,
                         start=(i == 0), stop=(i == npass - 1))
    nc.scalar.activation(dst_ap, bank[:], AF.Identity, bias=bias, scale=1.0)


def _w_scan(nc, lif, w_buf, z0, a_fn, nt=T, sliced=False):
    """w_t = (min(w_{t-1},1) - (w_{t-1}>=1))*0.5 + a_t via the custom op.
    in1 must keep >=2 free dims (STT encoding; the TTSS form runs ~10x slower)."""
    for t in range(nt):
        if sliced:
            out = w_buf[:, t:t + 1, :]
            in0 = z0[:] if t == 0 else w_buf[:, t - 1:t, :]
        else:
            out = w_buf[:, t]
            in0 = z0[:] if t == 0 else w_buf[:, t - 1]
        nc.vector._custom_dve(lif, out=out, in0=in0, in1=a_fn(t), s0=0.5)


def _layer_norm(nc, ps, sb, u, sq_buf, gamma_col, beta_col, out_fn,
                ones_col, ones_row, eps_col, csl, W):
    """LN over features (partitions x KC chunks) on a column slice csl of
    width W. u: [128, KC, TR] fp32; out_fn(kc) -> dst AP for that slice."""
    for kc in range(KC):
        nc.scalar.activation(sq_buf[:, kc, csl], u[:, kc, csl], AF.Square)
    ps_m = ps.tile([1, W], F32, tag="st", name="ps_m", bufs=2)
    ps_q = ps.tile([1, W], F32, tag="st", name="ps_q", bufs=2)
    for kc in range(KC):
        nc.tensor.matmul(ps_m[:], ones_col[:], u[:, kc, csl],
                         start=(kc == 0), stop=(kc == KC - 1))
    for kc in range(KC):
        nc.tensor.matmul(ps_q[:], ones_col[:], sq_buf[:, kc, csl],
                         start=(kc == 0), stop=(kc == KC - 1))
    m_sb = sb.tile([1, W], F32, name="m_sb", tag="m_sb", bufs=2)
    q_sb = sb.tile([1, W], F32, name="q_sb", tag="q_sb", bufs=2)
    nc.scalar.mul(m_sb[:], ps_m[:], 1.0 / D)
    nc.scalar.mul(q_sb[:], ps_q[:], 1.0 / D)
    ve = sb.tile([1, W], F32, name="ve", tag="ve", bufs=2)
    nc.vector.tensor_mul(out=ve[:], in0=m_sb[:], in1=m_sb[:])
    nc.vector.tensor_sub(out=ve[:], in0=q_sb[:], in1=ve[:])
    # rstd = 1/sqrt(var+eps): ACT sqrt (eps via bias) + fast reciprocal
    r0 = sb.tile([1, W], F32, name="r0", tag="r0", bufs=2)
    nc.scalar.activation(r0[:], ve[:], AF.Sqrt, bias=eps_col[:])
    nc.vector.reciprocal_approx_fast(r0[:], r0[:])
    pb_m = ps.tile([128, W], F32, tag="bc", name="pb_m", bufs=2)
    pb_r = ps.tile([128, W], F32, tag="bc", name="pb_r", bufs=2)
    nc.tensor.matmul(pb_m[:], ones_row[:], m_sb[:], start=True, stop=True)
    nc.tensor.matmul(pb_r[:], ones_row[:], r0[:], start=True, stop=True)
    for kc in range(KC):
        o = out_fn(kc)
        nc.vector.tensor_sub(out=o, in0=u[:, kc, csl], in1=pb_m[:])
        nc.vector.tensor_mul(out=o, in0=o, in1=pb_r[:])
        nc.vector.tensor_scalar(out=o, in0=o, scalar1=gamma_col(kc),
                                scalar2=beta_col(kc), op0=OP.mult, op1=OP.add)


def build_blocks():
    lif = _get_lif_op()
    nc = bacc.Bacc("TRN2", target_bir_lowering=False)
    x0_d = nc.dram_tensor("x0", [128, KC, TR], F32, kind="ExternalInput")
    w16_d = nc.dram_tensor("w16", [L, 128, W16], F16, kind="ExternalInput")
    w32_d = nc.dram_tensor("w32", [L, 128, WS], F32, kind="ExternalInput")
    h_d = nc.dram_tensor("h_out", [128, KC, TR], F32, kind="ExternalOutput")

    with tile.TileContext(nc) as tc:
        with tc.tile_pool(name="wp", bufs=2) as wp, \
             tc.tile_pool(name="sb", bufs=1) as sb, \
             tc.tile_pool(name="ps", bufs=1, space="PSUM") as ps:

            ones_col = sb.tile([128, 1], F32)
            ones_row = sb.tile([1, 128], F32)
            eps_col = sb.tile([1, 1], F32)
            nc.vector.memset(ones_col[:], 1.0)
            nc.vector.memset(ones_row[:], 1.0)
            nc.vector.memset(eps_col[:], EPS)

            x_cur = sb.tile([128, KC, TR], F32)
            nc.sync.dma_start(x_cur[:], x0_d.ap()[:])

            xh = sb.tile([128, KC, TR], F16)
            xl = sb.tile([128, KC, TR], F16)
            aga = sb.tile([128, 6, TR // 2], F32)
            agb = sb.tile([128, 6, TR // 2], F32)
            wg_buf = sb.tile([128, T, 6, R], F32)
            s_buf = sb.tile([128, T, 6, R], F16)
            kv_buf = sb.tile([128, T, KC, R], F16)
            h_buf = sb.tile([128, T, KC, R], F32)
            hh16 = sb.tile([128, T, KC, R], F16)
            hl16 = sb.tile([128, T, KC, R], F16)
            rhh = sb.tile([128, T, KC, R], F16)
            rhl = sb.tile([128, T, KC, R], F16)
            at_buf = sb.tile([128, KC, TR], F32)
            u_buf = sb.tile([128, KC, TR], F32)
            sq_buf = sb.tile([128, KC, TR], F32)
            x1_buf = sb.tile([128, KC, TR], F32)
            x1h = sb.tile([128, KC, TR], F16)
            x1l = sb.tile([128, KC, TR], F16)
            a1a = sb.tile([128, FC, TR // 2], F32)
            a1b = sb.tile([128, FC, TR // 2], F32)
            w1_buf = sb.tile([128, T, FC, R], F32)
            s1_buf = sb.tile([128, T, FC, R], F16)
            a2a = sb.tile([128, KC, TR // 2], F32)
            a2b = sb.tile([128, KC, TR // 2], F32)
            w2_buf = sb.tile([128, T, KC, R], F32)
            s2_buf = sb.tile([128, T, KC, R], F16)
            zg = sb.tile([128, 6, R], F32)
            zh = sb.tile([128, KC, R], F32)
            z1 = sb.tile([128, FC, R], F32)
            nc.vector.memset(zg[:], 0.0)
            nc.vector.memset(zh[:], 0.0)
            nc.vector.memset(z1[:], 0.0)

            wl16 = [wp.tile([128, W16], F16, tag="w16", name=f"w16_{i}")
                    for i in range(L)]
            wl32 = [wp.tile([128, WS], F32, tag="w32", name=f"w32_{i}")
                    for i in range(L)]
            for l in range(L):
                nc.sync.dma_start(wl16[l][:], w16_d.ap()[l])
                nc.sync.dma_start(wl32[l][:], w32_d.ap()[l])

            def tile16(wl, base, idx):
                off = base + idx * 128
                return wl[:, off:off + 128]

            for l in range(L):
                w6, w2c = wl16[l], wl32[l]

                if l == 0:
                    # layer 0: x is 0/1 spikes (xl == 0 exactly)
                    nc.vector.tensor_copy(out=xh[:], in_=x_cur[:])
                    nc.vector.tensor_sub(out=xl[:], in0=x_cur[:], in1=xh[:])

                # --- gates: 6 banks x (xh@Wh + xl@Wh + xh@Wl), T-split halves ---
                HT = TR // 2
                for half, agx in ((0, aga), (1, agb)):
                    sl = slice(half * HT, (half + 1) * HT)
                    for g in range(3):
                        for hf in range(KC):
                            bank = g * KC + hf
                            passes = []
                            for kc in range(KC):
                                wh = tile16(w6, GH_OFF, bank * KC + kc)
                                wlo = tile16(w6, GL_OFF, bank * KC + kc)
                                passes.append((wh, xh[:, kc, sl]))
                                if l > 0:  # layer 0: x is 0/1, xl == 0
                                    passes.append((wh, xl[:, kc, sl]))
                                passes.append((wlo, xh[:, kc, sl]))
                            _mm16(nc, ps, passes, agx[:, bank, :],
                                  name=f"g{half}{bank}", free=HT)

                # --- gate LIF scan; per half: spikes, kv, h-recurrence,
                #     rh (as hi/lo via h split: r in {0,1}), Wo matmuls ---
                def ag_src(t):
                    agx = aga if t < 8 else agb
                    tt = t % 8
                    return agx[:, :, tt * R:(tt + 1) * R]

                for t in range(T):
                    nc.vector._custom_dve(
                        lif, out=wg_buf[:, t],
                        in0=(zg[:] if t == 0 else wg_buf[:, t - 1]),
                        in1=ag_src(t), s0=0.5)
                    if t == 7 or t == 15:
                        half = 0 if t == 7 else 1
                        hh = slice(t - 7, t + 1)
                        nc.vector.tensor_scalar(
                            out=s_buf[:, hh], in0=wg_buf[:, hh], scalar1=1.0,
                            scalar2=None, op0=OP.is_ge)
                        nc.vector.tensor_mul(
                            out=kv_buf[:, hh], in0=s_buf[:, hh, 2:4, :],
                            in1=s_buf[:, hh, 4:6, :])
                        for th in range(t - 7, t + 1):
                            nc.vector.scalar_tensor_tensor(
                                out=h_buf[:, th],
                                in0=(zh[:] if th == 0 else h_buf[:, th - 1]),
                                scalar=0.9, in1=kv_buf[:, th],
                                op0=OP.mult, op1=OP.add)
                        # h hi/lo split; rh_hi = r*h_hi, rh_lo = r*h_lo
                        # (exact: r is 0/1)
                        nc.vector.tensor_copy(out=hh16[:, hh], in_=h_buf[:, hh])
                        nc.vector.tensor_sub(out=hl16[:, hh],
                                             in0=h_buf[:, hh], in1=hh16[:, hh])
                        nc.vector.tensor_mul(out=rhh[:, hh],
                                             in0=s_buf[:, hh, 0:2, :],
                                             in1=hh16[:, hh])
                        nc.vector.tensor_mul(out=rhl[:, hh],
                                             in0=s_buf[:, hh, 0:2, :],
                                             in1=hl16[:, hh])
                        for hf in range(KC):
                            passes = []
                            for kc in range(KC):
                                wh = tile16(w6, WOH_OFF, hf * KC + kc)
                                wlo = tile16(w6, WOL_OFF, hf * KC + kc)
                                passes += [(wh, rhh[:, hh, kc, :]),
                                           (wh, rhl[:, hh, kc, :]),
                                           (wlo, rhh[:, hh, kc, :])]
                            _mm16(nc, ps, passes,
                                  at_buf[:, hf, half * HT:(half + 1) * HT],
                                  name=f"wo{half}{hf}", free=HT)

                # --- LN1(x + attn) -> x1 and FFN mm1, pipelined per half ---
                for half, a1x in ((0, a1a), (1, a1b)):
                    sl = slice(half * HT, (half + 1) * HT)
                    for kc in range(KC):
                        nc.vector.tensor_add(out=u_buf[:, kc, sl],
                                             in0=x_cur[:, kc, sl],
                                             in1=at_buf[:, kc, sl])
                    _layer_norm(
                        nc, ps, sb, u_buf, sq_buf,
                        lambda kc: w2c[:, LN_OFF + kc:LN_OFF + kc + 1],
                        lambda kc: w2c[:, LN_OFF + 2 + kc:LN_OFF + 2 + kc + 1],
                        lambda kc: x1_buf[:, kc, sl],
                        ones_col, ones_row, eps_col, sl, HT)
                    nc.vector.tensor_copy(out=x1h[:, :, sl], in_=x1_buf[:, :, sl])
                    nc.vector.tensor_sub(out=x1l[:, :, sl], in0=x1_buf[:, :, sl],
                                         in1=x1h[:, :, sl])
                    for mf in range(FC):
                        passes = []
                        for kc in range(KC):
                            wh = tile16(w6, W1H_OFF, mf * KC + kc)
                            wlo = tile16(w6, W1L_OFF, mf * KC + kc)
                            passes += [(wh, x1h[:, kc, sl]), (wh, x1l[:, kc, sl]),
                                       (wlo, x1h[:, kc, sl])]
                        _mm16(nc, ps, passes, a1x[:, mf, :],
                              bias=w2c[:, B1_OFF + mf:B1_OFF + mf + 1],
                              name=f"f{half}{mf}", free=HT)

                # --- LIF1, spikes per half ---
                def a1_src(t):
                    a1x = a1a if t < 8 else a1b
                    tt = t % 8
                    return a1x[:, :, tt * R:(tt + 1) * R]

                for t in range(T):
                    nc.vector._custom_dve(
                        lif, out=w1_buf[:, t],
                        in0=(z1[:] if t == 0 else w1_buf[:, t - 1]),
                        in1=a1_src(t), s0=0.5)
                    if t == 7 or t == 15:
                        hh = slice(t - 7, t + 1)
                        nc.vector.tensor_scalar(
                            out=s1_buf[:, hh], in0=w1_buf[:, hh], scalar1=1.0,
                            scalar2=None, op0=OP.is_ge)

                # --- mm2 (+b2): s1 exact fp16, 2 passes per K chunk, T-split ---
                for half, a2x in ((0, a2a), (1, a2b)):
                    tsl = slice(half * 8, (half + 1) * 8)
                    for mh in range(KC):
                        passes = []
                        for kc8 in range(FC):
                            passes += [
                                (tile16(w6, W2H_OFF, mh * FC + kc8),
                                 s1_buf[:, tsl, kc8, :]),
                                (tile16(w6, W2L_OFF, mh * FC + kc8),
                                 s1_buf[:, tsl, kc8, :]),
                            ]
                        _mm16(nc, ps, passes, a2x[:, mh, :],
                              bias=w2c[:, B2_OFF + mh:B2_OFF + mh + 1],
                              name=f"m2{half}{mh}", free=HT)

                # --- LIF2, spikes per half ---
                def a2_src(t):
                    a2x = a2a if t < 8 else a2b
                    tt = t % 8
                    return a2x[:, :, tt * R:(tt + 1) * R]

                for t in range(T):
                    nc.vector._custom_dve(
                        lif, out=w2_buf[:, t],
                        in0=(zh[:] if t == 0 else w2_buf[:, t - 1]),
                        in1=a2_src(t), s0=0.5)
                    if t == 7 or t == 15:
                        hh = slice(t - 7, t + 1)
                        nc.vector.tensor_scalar(
                            out=s2_buf[:, hh], in0=w2_buf[:, hh], scalar1=1.0,
                            scalar2=None, op0=OP.is_ge)

                # --- LN2(x1 + s2) -> x_cur, per half ---
                for half in (0, 1):
                    sl = slice(half * HT, (half + 1) * HT)
                    tsl = slice(half * 8, (half + 1) * 8)
                    for kc in range(KC):
                        nc.vector.tensor_add(out=u_buf[:, kc, sl],
                                             in0=x1_buf[:, kc, sl],
                                             in1=s2_buf[:, tsl, kc, :])
                    _layer_norm(
                        nc, ps, sb, u_buf, sq_buf,
                        lambda kc: w2c[:, LN_OFF + 4 + kc:LN_OFF + 4 + kc + 1],
                        lambda kc: w2c[:, LN_OFF + 6 + kc:LN_OFF + 6 + kc + 1],
                        lambda kc: x_cur[:, kc, sl],
                        ones_col, ones_row, eps_col, sl, HT)
                    if l + 1 < L:
                        nc.vector.tensor_copy(out=xh[:, :, sl],
                                              in_=x_cur[:, :, sl])
                        nc.vector.tensor_sub(out=xl[:, :, sl],
                                             in0=x_cur[:, :, sl],
                                             in1=xh[:, :, sl])

            nc.sync.dma_start(h_d.ap()[:], x_cur[:])
    nc.compile()
    return nc


def build_head():
    lif = _get_lif_op()
    nc = bacc.Bacc("TRN2", target_bir_lowering=False)
    hh_d = nc.dram_tensor("hTh", [128, KC, TN], F16, kind="ExternalInput")
    hl_d = nc.dram_tensor("hTl", [128, KC, TN], F16, kind="ExternalInput")
    wh_d = nc.dram_tensor("wouth", [128, KC * VCH * 128], F16, kind="ExternalInput")
    wl_d = nc.dram_tensor("woutl", [128, KC * VCH * 128], F16, kind="ExternalInput")
    b_d = nc.dram_tensor("boutp", [128, VCH], F32, kind="ExternalInput")
    o_d = nc.dram_tensor("out_sh", [VCH, 128, N], F32, kind="ExternalOutput")

    with tile.TileContext(nc) as tc:
        with tc.tile_pool(name="sb", bufs=1) as sb, \
             tc.tile_pool(name="ab", bufs=2) as ab, \
             tc.tile_pool(name="wb", bufs=2) as wb, \
             tc.tile_pool(name="ob", bufs=2) as ob, \
             tc.tile_pool(name="ps", bufs=1, space="PSUM") as ps:

            hTh = sb.tile([128, KC, TN], F16)
            hTl = sb.tile([128, KC, TN], F16)
            wouth = sb.tile([128, KC * VCH * 128], F16)
            woutl = sb.tile([128, KC * VCH * 128], F16)
            boutp = sb.tile([128, VCH], F32)
            z0 = sb.tile([128, 2, 128], F32)
            nc.sync.dma_start(hTh[:], hh_d.ap()[:])
            nc.sync.dma_start(hTl[:], hl_d.ap()[:])
            nc.sync.dma_start(boutp[:], b_d.ap()[:])
            # wout in (kc, quarter) pieces so early chunks start sooner
            QW = VCH // 4 * 128
            for kc in range(KC):
                for q in range(4):
                    off = kc * VCH * 128 + q * QW
                    nc.sync.dma_start(wouth[:, off:off + QW],
                                      wh_d.ap()[:, off:off + QW])
                    nc.sync.dma_start(woutl[:, off:off + QW],
                                      wl_d.ap()[:, off:off + QW])
            nc.vector.memset(z0[:], 0.0)

            NB = TN // 512
            for c in range(VCH):
                a_buf = ab.tile([128, T, 2, 128], F32, tag="a", name=f"a{c}")
                banks = [ps.tile([128, 512], F32, tag="mm", name=f"b{c}_{fb}",
                                 bufs=8) for fb in range(NB)]
                # weight-stationary: one LDWEIGHTS per weight tile, stream all
                # 8 free-blocks through it; each bank accumulates its 6 passes.
                wpasses = []
                for kc in range(KC):
                    off = (kc * VCH + c) * 128
                    wh_t = wouth[:, off:off + 128]
                    wl_t = woutl[:, off:off + 128]
                    wpasses += [(wh_t, kc, 0), (wh_t, kc, 1), (wl_t, kc, 0)]
                for pi, (wt, kc, lo) in enumerate(wpasses):
                    hsrc = hTl if lo else hTh
                    for fb in range(NB):
                        nc.tensor.matmul(
                            banks[fb][:], wt,
                            hsrc[:, kc, fb * 512:(fb + 1) * 512],
                            start=(pi == 0), stop=(pi == len(wpasses) - 1))
                for fb in range(NB):
                    nc.scalar.activation(
                        a_buf[:, 2 * fb:2 * fb + 2, :, :], banks[fb][:],
                        AF.Identity, bias=boutp[:, c:c + 1], scale=1.0)

                w_buf = ob.tile([128, T, 2, 128], F32, tag="w", name=f"w{c}")
                s_b = ob.tile([128, T, 2, 128], F32, tag="s", name=f"s{c}")
                for t in range(T):
                    nc.vector._custom_dve(
                        lif, out=w_buf[:, t],
                        in0=(z0[:] if t == 0 else w_buf[:, t - 1]),
                        in1=a_buf[:, t], s0=0.5)
                    if t == 7 or t == 15:
                        hh = slice(t - 7, t + 1)
                        nc.vector.tensor_scalar(
                            out=s_b[:, hh], in0=w_buf[:, hh], scalar1=1.0,
                            scalar2=None, op0=OP.is_ge)
                t8 = ob.tile([128, 8, 2, 128], F32, tag="t8", name=f"t8{c}")
                nc.gpsimd.tensor_add(out=t8[:], in0=s_b[:, 0:8], in1=s_b[:, 8:16])
                t4 = ob.tile([128, 4, 2, 128], F32, tag="t4", name=f"t4{c}")
                nc.gpsimd.tensor_add(out=t4[:], in0=t8[:, 0:4], in1=t8[:, 4:8])
                t2 = ob.tile([128, 2, 2, 128], F32, tag="t2", name=f"t2{c}")
                nc.vector.tensor_add(out=t2[:], in0=t4[:, 0:2], in1=t4[:, 2:4])
                acc = ob.tile([128, N], F32, tag="acc", name=f"acc{c}")
                nc.vector.tensor_add(out=acc[:], in0=t2[:, 0], in1=t2[:, 1])
                nc.sync.dma_start(o_d.ap()[c], acc[:])
    nc.compile()
    return nc


_CACHE = {}
TRACE = False
LAST = {}


def _run(nc, in_maps, key):
    import tempfile

    if TRACE:
        td = tempfile.mkdtemp(prefix=f"bkt_{key}_")
        res = run_bass_kernel_spmd(nc, in_maps, core_ids=list(range(NCORE)),
                                   trace=True, tmpdir=td)
        LAST[key] = (res, td)
        return res
    return run_bass_kernel_spmd(nc, in_maps, core_ids=list(range(NCORE)))


def _get_programs():
    if "blocks" not in _CACHE:
        _CACHE["blocks"] = build_blocks()
        _CACHE["head"] = build_head()
    return _CACHE["blocks"], _CACHE["head"]


def _pack_weights(Wr, Wk, Wv, Wo, W1, b1, W2, b2, g1, be1, g2, be2):
    w16 = np.zeros((L, 128, W16), np.float16)
    w32 = np.zeros((L, 128, WS), np.float32)
    for l in range(L):
        his, los = [], []

        def add(mat):  # mat [K, M] fp32 -> hi/lo tiles
            hi, lo = _split16(mat)
            his.append(hi)
            los.append(lo)

        for Wg in (Wr, Wk, Wv):
            for hf in range(KC):
                for kc in range(KC):
                    add(0.5 * Wg[l][kc * 128:(kc + 1) * 128,
                                    hf * 128:(hf + 1) * 128])
        gh = np.concatenate(his, axis=1)
        gl = np.concatenate(los, axis=1)
        his, los = [], []
        for hf in range(KC):
            for kc in range(KC):
                add(Wo[l][kc * 128:(kc + 1) * 128, hf * 128:(hf + 1) * 128])
        woh = np.concatenate(his, axis=1)
        wol = np.concatenate(los, axis=1)
        his, los = [], []
        for mf in range(FC):
            for kc in range(KC):
                add(0.5 * W1[l][kc * 128:(kc + 1) * 128, mf * 128:(mf + 1) * 128])
        w1h = np.concatenate(his, axis=1)
        w1l = np.concatenate(los, axis=1)
        his, los = [], []
        for mh in range(KC):
            for kc8 in range(FC):
                add(0.5 * W2[l][kc8 * 128:(kc8 + 1) * 128,
                                mh * 128:(mh + 1) * 128])
        w2h = np.concatenate(his, axis=1)
        w2l = np.concatenate(los, axis=1)
        w16[l] = np.concatenate([gh, gl, woh, wol, w1h, w1l, w2h, w2l], axis=1)
        w32[l] = np.concatenate([
            0.5 * b1[l].reshape(FC, 128).T,
            0.5 * b2[l].reshape(KC, 128).T,
            g1[l].reshape(KC, 128).T, be1[l].reshape(KC, 128).T,
            g2[l].reshape(KC, 128).T, be2[l].reshape(KC, 128).T,
        ], axis=1)
    return np.ascontiguousarray(w16), np.ascontiguousarray(w32)


def kernel(input_ids, token_embedding, pos_embedding, noise, unif,
           Wr, Wk, Wv, Wo, W1, b1, W2, b2, ln1_g, ln1_b, ln2_g, ln2_b,
           Wout, bout):
    input_ids = np.asarray(input_ids)
    f32 = lambda a: np.asarray(a, dtype=np.float32)
    token_embedding, pos_embedding, noise, unif = map(
        f32, (token_embedding, pos_embedding, noise, unif))
    Wr, Wk, Wv, Wo, W1, b1, W2, b2 = map(f32, (Wr, Wk, Wv, Wo, W1, b1, W2, b2))
    ln1_g, ln1_b, ln2_g, ln2_b, Wout, bout = map(
        f32, (ln1_g, ln1_b, ln2_g, ln2_b, Wout, bout))

    nc_blocks, nc_head = _get_programs()

    spikes = _encode_spikes(input_ids, token_embedding, pos_embedding, noise, unif)
    sp = spikes.reshape(T, NCORE, R, KC, 128)          # (t, core, r, kc, p)
    x0 = np.ascontiguousarray(sp.transpose(1, 4, 3, 0, 2)).reshape(NCORE, 128, KC, TR)
    w16, w32 = _pack_weights(Wr, Wk, Wv, Wo, W1, b1, W2, b2,
                             ln1_g, ln1_b, ln2_g, ln2_b)
    in1 = [{"x0": x0[c], "w16": w16, "w32": w32} for c in range(NCORE)]
    res1 = _run(nc_blocks, in1, "blocks")
    ho = np.stack([res1.results[c]["h_out"].reshape(128, KC, T, R)
                   for c in range(NCORE)])
    hT = np.ascontiguousarray(ho.transpose(1, 2, 3, 0, 4)).reshape(128, KC, TN)
    hTh, hTl = _split16(hT)

    Wp = np.zeros((D, VPAD), np.float32)
    Wp[:, :V] = 0.5 * Wout
    Wph, Wpl = _split16(Wp)
    bp = np.zeros((VPAD,), np.float32)
    bp[:V] = 0.5 * bout
    in2 = []
    for c in range(NCORE):
        def shard(Wx):
            w = Wx[:, c * VSH:(c + 1) * VSH].reshape(KC, 128, VCH, 128)
            return np.ascontiguousarray(
                w.transpose(1, 0, 2, 3)).reshape(128, KC * VCH * 128)
        bsh = np.ascontiguousarray(bp[c * VSH:(c + 1) * VSH].reshape(VCH, 128).T)
        in2.append({"hTh": hTh, "hTl": hTl, "wouth": shard(Wph),
                    "woutl": shard(Wpl), "boutp": bsh})
    res2 = _run(nc_head, in2, "head")
    out_sh = np.stack([res2.results[c]["out_sh"] for c in range(NCORE)])
    out = out_sh.reshape(VPAD, N)[:V]
    out = np.ascontiguousarray(out.T).reshape(B, S, V).astype(np.float32)
    return out


# revision 18
# speedup vs baseline: 1.0635x; 1.0635x over previous
"""Trainium2 Bass kernel for nn_AdvancedSpikingChatModel.

Model: spike-encode embeddings -> 6 spiking-transformer blocks (LIF gates +
decaying linear-attention recurrence over T=16) -> LIF output head with
spike-count accumulation over V=32000 vocab.

Strategy (8 NeuronCores, SPMD, two launches):
  Launch 1 (blocks): data-parallel over the 256 folded (b,s) rows, 32/core.
    Features on partitions, (t, row) on the free dim; weights stationary.
  Launch 2 (head): vocab-parallel, 4096 padded cols/core, all 256 rows.

Precision: matmuls run as fp16 hi/lo split passes (x@W = xh@Wh + xl@Wh +
xh@Wl accumulated in fp32 PSUM; dropped xl@Wl term ~2^-22) — fp32-grade
results at the PE's fp16 rate (fp32 matmuls cost ~2.6x on TRN2). The LIF
threshold compare (v >= 1) makes anything coarser (bf16/fp32r) flip spikes.
Spike matrices (0/1) are exact in fp16, so spike-side matmuls use 2 passes.

LIF decay 0.5 folded into weights: w' = 0.5*(min(w,1) - (w>=1)) + a, emitted
as ONE custom DVE op per step; spikes s = (w >= 1) recovered in one batched
GPSIMD pass per scan; spike counts via add-tree (GPSIMD + DVE).
"""

import numpy as np

import concourse.mybir as mybir
import concourse.tile as tile
from concourse import bacc
from concourse.bass_utils import run_bass_kernel_spmd

F32 = mybir.dt.float32
F16 = mybir.dt.float16
OP = mybir.AluOpType
AF = mybir.ActivationFunctionType

B, S, D, T, L, F, V = 2, 128, 256, 16, 6, 1024, 32000
N = B * S
NCORE = 8
R = N // NCORE       # 32 rows/core in launch 1
TR = T * R           # 512
KC = D // 128
FC = F // 128
VPAD = 32768
VSH = VPAD // NCORE  # 4096
VCH = VSH // 128     # 32 chunks
TN = T * N           # 4096
EPS = 1e-5

# fp16 weight slab offsets (fp16 words per partition, per layer)
GH_OFF = 0
GL_OFF = GH_OFF + 12 * 128
WOH_OFF = GL_OFF + 12 * 128
WOL_OFF = WOH_OFF + 4 * 128
W1H_OFF = WOL_OFF + 4 * 128
W1L_OFF = W1H_OFF + 16 * 128
W2H_OFF = W1L_OFF + 16 * 128
W2L_OFF = W2H_OFF + 16 * 128
W16 = W2L_OFF + 16 * 128
# fp32 smalls: b1(8) b2(2) ln(8)
B1_OFF = 0
B2_OFF = 8
LN_OFF = 10
WS = 18

_LIF_OP = None


def _get_lif_op():
    """Register the fused LIF step as a local custom DVE op:
    out = (min(w,1) - (w>=1))*0.5 + a."""
    global _LIF_OP
    if _LIF_OP is not None:
        return _LIF_OP
    from concourse.dve_spec import Spec, Src0, Src1, C0, One, minn, lower
    from concourse.dve_ops import (
        DveOp, OPS, _SUB_OPCODE_FOR_NAME, CUSTOM_DVE_SPECS)
    from concourse.dve_uop import DveOpSpec

    name = "LIF_STEP_ANT"
    if name not in _SUB_OPCODE_FOR_NAME:
        body = (minn(Src0, One) - (Src0 >= One)) * C0 + Src1
        spec = Spec(
            body=body,
            reference=lambda in0, in1, s0, s1, imm2:
                (np.minimum(in0, 1.0) - (in0 >= 1.0)) * s0 + in1,
        )
        op = DveOp(name, spec, subdim=False, uops_sha={})
        row = 1 + len(OPS)
        OPS.append(op)
        _SUB_OPCODE_FOR_NAME[name] = row
        CUSTOM_DVE_SPECS[name] = spec
        for ver in ("v3",):
            s = DveOpSpec(name=name, opcode=row, uops=lower(spec, ver=ver),
                          rd1_en=True)
            op.uops_sha[ver] = s.sha(ver)
        _LIF_OP = op
    else:
        _LIF_OP = next(o for o in OPS if o.name == name)
    return _LIF_OP


def _sigmoid(x):
    return 1.0 / (1.0 + np.exp(-x))


def _encode_spikes(input_ids, token_embedding, pos_embedding, noise, unif):
    """Host-side rate coding; (0.7*rate + 0.3*temp > 0.5) == rate exactly."""
    emb = token_embedding[input_ids] + pos_embedding[None, :S]
    p = np.clip(_sigmoid(emb) * 0.8 + 0.1 + noise * 0.05, 0.0, 1.0)
    return (unif < p[None]).astype(np.float32)


def _split16(x):
    hi = x.astype(np.float16)
    lo = (x - hi.astype(np.float32)).astype(np.float16)
    return hi, lo


def _mm16(nc, ps, passes, dst_ap, bias=0.0, name="mmb", free=512):
    """Accumulate fp16 matmul passes into one PSUM bank, ACT-copy(+bias) out."""
    bank = ps.tile([128, free], F32, tag="mm", name=name, bufs=4)
    npass = len(passes)
    for i, (lhsT, rhs) in enumerate(passes):
        nc.tensor.matmul(bank[:], lhsT, rhs,
                         start=(i == 0), stop=(i == npass - 1))
    nc.scalar.activation(dst_ap, bank[:], AF.Identity, bias=bias, scale=1.0)


def _w_scan(nc, lif, w_buf, z0, a_fn, nt=T, sliced=False):
    """w_t = (min(w_{t-1},1) - (w_{t-1}>=1))*0.5 + a_t via the custom op.
    in1 must keep >=2 free dims (STT encoding; the TTSS form runs ~10x slower)."""
    for t in range(nt):
        if sliced:
            out = w_buf[:, t:t + 1, :]
            in0 = z0[:] if t == 0 else w_buf[:, t - 1:t, :]
        else:
            out = w_buf[:, t]
            in0 = z0[:] if t == 0 else w_buf[:, t - 1]
        nc.vector._custom_dve(lif, out=out, in0=in0, in1=a_fn(t), s0=0.5)


def _layer_norm(nc, ps, sb, u, sq_buf, gamma_col, beta_col, out_fn,
                ones_col, ones_row, eps_col, csl, W):
    """LN over features (partitions x KC chunks) on a column slice csl of
    width W. u: [128, KC, TR] fp32; out_fn(kc) -> dst AP for that slice."""
    for kc in range(KC):
        nc.scalar.activation(sq_buf[:, kc, csl], u[:, kc, csl], AF.Square)
    ps_m = ps.tile([1, W], F32, tag="st", name="ps_m", bufs=2)
    ps_q = ps.tile([1, W], F32, tag="st", name="ps_q", bufs=2)
    for kc in range(KC):
        nc.tensor.matmul(ps_m[:], ones_col[:], u[:, kc, csl],
                         start=(kc == 0), stop=(kc == KC - 1))
    for kc in range(KC):
        nc.tensor.matmul(ps_q[:], ones_col[:], sq_buf[:, kc, csl],
                         start=(kc == 0), stop=(kc == KC - 1))
    m_sb = sb.tile([1, W], F32, name="m_sb", tag="m_sb", bufs=2)
    q_sb = sb.tile([1, W], F32, name="q_sb", tag="q_sb", bufs=2)
    nc.scalar.mul(m_sb[:], ps_m[:], 1.0 / D)
    nc.scalar.mul(q_sb[:], ps_q[:], 1.0 / D)
    ve = sb.tile([1, W], F32, name="ve", tag="ve", bufs=2)
    nc.vector.tensor_mul(out=ve[:], in0=m_sb[:], in1=m_sb[:])
    nc.vector.tensor_sub(out=ve[:], in0=q_sb[:], in1=ve[:])
    # rstd = 1/sqrt(var+eps): ACT sqrt (eps via bias) + fast reciprocal
    r0 = sb.tile([1, W], F32, name="r0", tag="r0", bufs=2)
    nc.scalar.activation(r0[:], ve[:], AF.Sqrt, bias=eps_col[:])
    nc.vector.reciprocal_approx_fast(r0[:], r0[:])
    pb_m = ps.tile([128, W], F32, tag="bc", name="pb_m", bufs=2)
    pb_r = ps.tile([128, W], F32, tag="bc", name="pb_r", bufs=2)
    nc.tensor.matmul(pb_m[:], ones_row[:], m_sb[:], start=True, stop=True)
    nc.tensor.matmul(pb_r[:], ones_row[:], r0[:], start=True, stop=True)
    for kc in range(KC):
        o = out_fn(kc)
        nc.vector.tensor_sub(out=o, in0=u[:, kc, csl], in1=pb_m[:])
        nc.vector.tensor_mul(out=o, in0=o, in1=pb_r[:])
        nc.vector.tensor_scalar(out=o, in0=o, scalar1=gamma_col(kc),
                                scalar2=beta_col(kc), op0=OP.mult, op1=OP.add)


def build_blocks():
    lif = _get_lif_op()
    nc = bacc.Bacc("TRN2", target_bir_lowering=False)
    x0_d = nc.dram_tensor("x0", [128, KC, TR], F32, kind="ExternalInput")
    w16_d = nc.dram_tensor("w16", [L, 128, W16], F16, kind="ExternalInput")
    w32_d = nc.dram_tensor("w32", [L, 128, WS], F32, kind="ExternalInput")
    h_d = nc.dram_tensor("h_out", [128, KC, TR], F32, kind="ExternalOutput")

    with tile.TileContext(nc) as tc:
        with tc.tile_pool(name="wp", bufs=2) as wp, \
             tc.tile_pool(name="sb", bufs=1) as sb, \
             tc.tile_pool(name="ps", bufs=1, space="PSUM") as ps:

            ones_col = sb.tile([128, 1], F32)
            ones_row = sb.tile([1, 128], F32)
            eps_col = sb.tile([1, 1], F32)
            nc.vector.memset(ones_col[:], 1.0)
            nc.vector.memset(ones_row[:], 1.0)
            nc.vector.memset(eps_col[:], EPS)

            x_cur = sb.tile([128, KC, TR], F32)
            nc.sync.dma_start(x_cur[:], x0_d.ap()[:])

            xh = sb.tile([128, KC, TR], F16)
            xl = sb.tile([128, KC, TR], F16)
            aga = sb.tile([128, 6, TR // 2], F32)
            agb = sb.tile([128, 6, TR // 2], F32)
            wg_buf = sb.tile([128, T, 6, R], F32)
            s_buf = sb.tile([128, T, 6, R], F16)
            kv_buf = sb.tile([128, T, KC, R], F16)
            h_buf = sb.tile([128, T, KC, R], F32)
            hh16 = sb.tile([128, T, KC, R], F16)
            hl16 = sb.tile([128, T, KC, R], F16)
            rhh = sb.tile([128, T, KC, R], F16)
            rhl = sb.tile([128, T, KC, R], F16)
            at_buf = sb.tile([128, KC, TR], F32)
            u_buf = sb.tile([128, KC, TR], F32)
            sq_buf = sb.tile([128, KC, TR], F32)
            x1_buf = sb.tile([128, KC, TR], F32)
            x1h = sb.tile([128, KC, TR], F16)
            x1l = sb.tile([128, KC, TR], F16)
            a1a = sb.tile([128, FC, TR // 2], F32)
            a1b = sb.tile([128, FC, TR // 2], F32)
            w1_buf = sb.tile([128, T, FC, R], F32)
            s1_buf = sb.tile([128, T, FC, R], F16)
            a2a = sb.tile([128, KC, TR // 2], F32)
            a2b = sb.tile([128, KC, TR // 2], F32)
            w2_buf = sb.tile([128, T, KC, R], F32)
            s2_buf = sb.tile([128, T, KC, R], F16)
            zg = sb.tile([128, 6, R], F32)
            zh = sb.tile([128, KC, R], F32)
            z1 = sb.tile([128, FC, R], F32)
            nc.vector.memset(zg[:], 0.0)
            nc.vector.memset(zh[:], 0.0)
            nc.vector.memset(z1[:], 0.0)

            wl16 = [wp.tile([128, W16], F16, tag="w16", name=f"w16_{i}")
                    for i in range(L)]
            wl32 = [wp.tile([128, WS], F32, tag="w32", name=f"w32_{i}")
                    for i in range(L)]
            for l in range(L):
                nc.sync.dma_start(wl16[l][:], w16_d.ap()[l])
                nc.sync.dma_start(wl32[l][:], w32_d.ap()[l])

            def tile16(wl, base, idx):
                off = base + idx * 128
                return wl[:, off:off + 128]

            for l in range(L):
                w6, w2c = wl16[l], wl32[l]

                if l == 0:
                    # layer 0: x is 0/1 spikes (xl == 0 exactly)
                    nc.vector.tensor_copy(out=xh[:], in_=x_cur[:])
                    nc.vector.tensor_sub(out=xl[:], in0=x_cur[:], in1=xh[:])

                # --- gates: 6 banks x (xh@Wh + xl@Wh + xh@Wl), T-split halves ---
                HT = TR // 2
                for half, agx in ((0, aga), (1, agb)):
                    sl = slice(half * HT, (half + 1) * HT)
                    for g in range(3):
                        for hf in range(KC):
                            bank = g * KC + hf
                            passes = []
                            for kc in range(KC):
                                wh = tile16(w6, GH_OFF, bank * KC + kc)
                                wlo = tile16(w6, GL_OFF, bank * KC + kc)
                                passes.append((wh, xh[:, kc, sl]))
                                if l > 0:  # layer 0: x is 0/1, xl == 0
                                    passes.append((wh, xl[:, kc, sl]))
                                passes.append((wlo, xh[:, kc, sl]))
                            _mm16(nc, ps, passes, agx[:, bank, :],
                                  name=f"g{half}{bank}", free=HT)

                # --- gate LIF scan; per half: spikes, kv, h-recurrence,
                #     rh (as hi/lo via h split: r in {0,1}), Wo matmuls ---
                def ag_src(t):
                    agx = aga if t < 8 else agb
                    tt = t % 8
                    return agx[:, :, tt * R:(tt + 1) * R]

                for t in range(T):
                    nc.vector._custom_dve(
                        lif, out=wg_buf[:, t],
                        in0=(zg[:] if t == 0 else wg_buf[:, t - 1]),
                        in1=ag_src(t), s0=0.5)
                    if t == 7 or t == 15:
                        half = 0 if t == 7 else 1
                        hh = slice(t - 7, t + 1)
                        nc.vector.tensor_scalar(
                            out=s_buf[:, hh], in0=wg_buf[:, hh], scalar1=1.0,
                            scalar2=None, op0=OP.is_ge)
                        nc.vector.tensor_mul(
                            out=kv_buf[:, hh], in0=s_buf[:, hh, 2:4, :],
                            in1=s_buf[:, hh, 4:6, :])
                        for th in range(t - 7, t + 1):
                            nc.vector.scalar_tensor_tensor(
                                out=h_buf[:, th],
                                in0=(zh[:] if th == 0 else h_buf[:, th - 1]),
                                scalar=0.9, in1=kv_buf[:, th],
                                op0=OP.mult, op1=OP.add)
                        # h hi/lo split; rh_hi = r*h_hi, rh_lo = r*h_lo
                        # (exact: r is 0/1)
                        nc.vector.tensor_copy(out=hh16[:, hh], in_=h_buf[:, hh])
                        nc.vector.tensor_sub(out=hl16[:, hh],
                                             in0=h_buf[:, hh], in1=hh16[:, hh])
                        nc.vector.tensor_mul(out=rhh[:, hh],
                                             in0=s_buf[:, hh, 0:2, :],
                                             in1=hh16[:, hh])
                        nc.vector.tensor_mul(out=rhl[:, hh],
                                             in0=s_buf[:, hh, 0:2, :],
                                             in1=hl16[:, hh])
                        for hf in range(KC):
                            passes = []
                            for kc in range(KC):
                                wh = tile16(w6, WOH_OFF, hf * KC + kc)
                                wlo = tile16(w6, WOL_OFF, hf * KC + kc)
                                passes += [(wh, rhh[:, hh, kc, :]),
                                           (wh, rhl[:, hh, kc, :]),
                                           (wlo, rhh[:, hh, kc, :])]
                            _mm16(nc, ps, passes,
                                  at_buf[:, hf, half * HT:(half + 1) * HT],
                                  name=f"wo{half}{hf}", free=HT)

                # --- LN1(x + attn) -> x1 and FFN mm1, pipelined per half ---
                for half, a1x in ((0, a1a), (1, a1b)):
                    sl = slice(half * HT, (half + 1) * HT)
                    for kc in range(KC):
                        nc.vector.tensor_add(out=u_buf[:, kc, sl],
                                             in0=x_cur[:, kc, sl],
                                             in1=at_buf[:, kc, sl])
                    _layer_norm(
                        nc, ps, sb, u_buf, sq_buf,
                        lambda kc: w2c[:, LN_OFF + kc:LN_OFF + kc + 1],
                        lambda kc: w2c[:, LN_OFF + 2 + kc:LN_OFF + 2 + kc + 1],
                        lambda kc: x1_buf[:, kc, sl],
                        ones_col, ones_row, eps_col, sl, HT)
                    nc.vector.tensor_copy(out=x1h[:, :, sl], in_=x1_buf[:, :, sl])
                    nc.vector.tensor_sub(out=x1l[:, :, sl], in0=x1_buf[:, :, sl],
                                         in1=x1h[:, :, sl])
                    for mf in range(FC):
                        passes = []
                        for kc in range(KC):
                            wh = tile16(w6, W1H_OFF, mf * KC + kc)
                            wlo = tile16(w6, W1L_OFF, mf * KC + kc)
                            passes += [(wh, x1h[:, kc, sl]), (wh, x1l[:, kc, sl]),
                                       (wlo, x1h[:, kc, sl])]
                        _mm16(nc, ps, passes, a1x[:, mf, :],
                              bias=w2c[:, B1_OFF + mf:B1_OFF + mf + 1],
                              name=f"f{half}{mf}", free=HT)

                # --- LIF1, spikes per half ---
                def a1_src(t):
                    a1x = a1a if t < 8 else a1b
                    tt = t % 8
                    return a1x[:, :, tt * R:(tt + 1) * R]

                for t in range(T):
                    nc.vector._custom_dve(
                        lif, out=w1_buf[:, t],
                        in0=(z1[:] if t == 0 else w1_buf[:, t - 1]),
                        in1=a1_src(t), s0=0.5)
                    if t == 7 or t == 15:
                        hh = slice(t - 7, t + 1)
                        nc.vector.tensor_scalar(
                            out=s1_buf[:, hh], in0=w1_buf[:, hh], scalar1=1.0,
                            scalar2=None, op0=OP.is_ge)

                # --- mm2 (+b2): s1 exact fp16, 2 passes per K chunk, T-split ---
                for half, a2x in ((0, a2a), (1, a2b)):
                    tsl = slice(half * 8, (half + 1) * 8)
                    for mh in range(KC):
                        passes = []
                        for kc8 in range(FC):
                            passes += [
                                (tile16(w6, W2H_OFF, mh * FC + kc8),
                                 s1_buf[:, tsl, kc8, :]),
                                (tile16(w6, W2L_OFF, mh * FC + kc8),
                                 s1_buf[:, tsl, kc8, :]),
                            ]
                        _mm16(nc, ps, passes, a2x[:, mh, :],
                              bias=w2c[:, B2_OFF + mh:B2_OFF + mh + 1],
                              name=f"m2{half}{mh}", free=HT)

                # --- LIF2, spikes per half ---
                def a2_src(t):
                    a2x = a2a if t < 8 else a2b
                    tt = t % 8
                    return a2x[:, :, tt * R:(tt + 1) * R]

                for t in range(T):
                    nc.vector._custom_dve(
                        lif, out=w2_buf[:, t],
                        in0=(zh[:] if t == 0 else w2_buf[:, t - 1]),
                        in1=a2_src(t), s0=0.5)
                    if t == 7 or t == 15:
                        hh = slice(t - 7, t + 1)
                        nc.vector.tensor_scalar(
                            out=s2_buf[:, hh], in0=w2_buf[:, hh], scalar1=1.0,
                            scalar2=None, op0=OP.is_ge)

                # --- LN2(x1 + s2) -> x_cur, per half ---
                for half in (0, 1):
                    sl = slice(half * HT, (half + 1) * HT)
                    tsl = slice(half * 8, (half + 1) * 8)
                    for kc in range(KC):
                        nc.vector.tensor_add(out=u_buf[:, kc, sl],
                                             in0=x1_buf[:, kc, sl],
                                             in1=s2_buf[:, tsl, kc, :])
                    _layer_norm(
                        nc, ps, sb, u_buf, sq_buf,
                        lambda kc: w2c[:, LN_OFF + 4 + kc:LN_OFF + 4 + kc + 1],
                        lambda kc: w2c[:, LN_OFF + 6 + kc:LN_OFF + 6 + kc + 1],
                        lambda kc: x_cur[:, kc, sl],
                        ones_col, ones_row, eps_col, sl, HT)
                    if l + 1 < L:
                        nc.vector.tensor_copy(out=xh[:, :, sl],
                                              in_=x_cur[:, :, sl])
                        nc.vector.tensor_sub(out=xl[:, :, sl],
                                             in0=x_cur[:, :, sl],
                                             in1=xh[:, :, sl])

            nc.sync.dma_start(h_d.ap()[:], x_cur[:])
    nc.compile()
    return nc


def build_head():
    lif = _get_lif_op()
    nc = bacc.Bacc("TRN2", target_bir_lowering=False)
    hh_d = nc.dram_tensor("hTh", [128, KC, TN], F16, kind="ExternalInput")
    hl_d = nc.dram_tensor("hTl", [128, KC, TN], F16, kind="ExternalInput")
    wh_d = nc.dram_tensor("wouth", [128, KC * VCH * 128], F16, kind="ExternalInput")
    wl_d = nc.dram_tensor("woutl", [128, KC * VCH * 128], F16, kind="ExternalInput")
    b_d = nc.dram_tensor("boutp", [128, VCH], F32, kind="ExternalInput")
    o_d = nc.dram_tensor("out_sh", [VCH, 128, N], F32, kind="ExternalOutput")

    with tile.TileContext(nc) as tc:
        with tc.tile_pool(name="sb", bufs=1) as sb, \
             tc.tile_pool(name="ab", bufs=2) as ab, \
             tc.tile_pool(name="wb", bufs=2) as wb, \
             tc.tile_pool(name="ob", bufs=2) as ob, \
             tc.tile_pool(name="ps", bufs=1, space="PSUM") as ps:

            hTh = sb.tile([128, KC, TN], F16)
            hTl = sb.tile([128, KC, TN], F16)
            wouth = sb.tile([128, KC * VCH * 128], F16)
            woutl = sb.tile([128, KC * VCH * 128], F16)
            boutp = sb.tile([128, VCH], F32)
            z0 = sb.tile([128, 2, 128], F32)
            nc.sync.dma_start(hTh[:], hh_d.ap()[:])
            nc.sync.dma_start(hTl[:], hl_d.ap()[:])
            nc.sync.dma_start(boutp[:], b_d.ap()[:])
            # wout in (kc, quarter) pieces so early chunks start sooner
            QW = VCH // 4 * 128
            for kc in range(KC):
                for q in range(4):
                    off = kc * VCH * 128 + q * QW
                    nc.sync.dma_start(wouth[:, off:off + QW],
                                      wh_d.ap()[:, off:off + QW])
                    nc.sync.dma_start(woutl[:, off:off + QW],
                                      wl_d.ap()[:, off:off + QW])
            nc.vector.memset(z0[:], 0.0)

            NB = TN // 512
            for c in range(VCH):
                a_buf = ab.tile([128, T, 2, 128], F32, tag="a", name=f"a{c}")
                banks = [ps.tile([128, 512], F32, tag="mm", name=f"b{c}_{fb}",
                                 bufs=8) for fb in range(NB)]
                # weight-stationary: one LDWEIGHTS per weight tile, stream all
                # 8 free-blocks through it; each bank accumulates its 6 passes.
                wpasses = []
                for kc in range(KC):
                    off = (kc * VCH + c) * 128
                    wh_t = wouth[:, off:off + 128]
                    wl_t = woutl[:, off:off + 128]
                    wpasses += [(wh_t, kc, 0), (wh_t, kc, 1), (wl_t, kc, 0)]
                for pi, (wt, kc, lo) in enumerate(wpasses):
                    hsrc = hTl if lo else hTh
                    for fb in range(NB):
                        nc.tensor.matmul(
                            banks[fb][:], wt,
                            hsrc[:, kc, fb * 512:(fb + 1) * 512],
                            start=(pi == 0), stop=(pi == len(wpasses) - 1))
                for fb in range(NB):
                    nc.scalar.activation(
                        a_buf[:, 2 * fb:2 * fb + 2, :, :], banks[fb][:],
                        AF.Identity, bias=boutp[:, c:c + 1], scale=1.0)

                w_buf = ob.tile([128, T, 2, 128], F32, tag="w", name=f"w{c}")
                for t in range(T):
                    nc.vector._custom_dve(
                        lif, out=w_buf[:, t],
                        in0=(z0[:] if t == 0 else w_buf[:, t - 1]),
                        in1=a_buf[:, t], s0=0.5)
                s_b = ob.tile([128, T, 2, 128], F32, tag="s", name=f"s{c}")
                nc.vector.tensor_scalar(
                    out=s_b[:], in0=w_buf[:], scalar1=1.0, scalar2=None,
                    op0=OP.is_ge)
                t8 = ob.tile([128, 8, 2, 128], F32, tag="t8", name=f"t8{c}")
                nc.gpsimd.tensor_add(out=t8[:], in0=s_b[:, 0:8], in1=s_b[:, 8:16])
                t4 = ob.tile([128, 4, 2, 128], F32, tag="t4", name=f"t4{c}")
                nc.vector.tensor_add(out=t4[:], in0=t8[:, 0:4], in1=t8[:, 4:8])
                t2 = ob.tile([128, 2, 2, 128], F32, tag="t2", name=f"t2{c}")
                nc.vector.tensor_add(out=t2[:], in0=t4[:, 0:2], in1=t4[:, 2:4])
                acc = ob.tile([128, N], F32, tag="acc", name=f"acc{c}")
                nc.vector.tensor_add(out=acc[:], in0=t2[:, 0], in1=t2[:, 1])
                nc.sync.dma_start(o_d.ap()[c], acc[:])
    nc.compile()
    return nc


_CACHE = {}
TRACE = False
LAST = {}


def _run(nc, in_maps, key):
    import tempfile

    if TRACE:
        td = tempfile.mkdtemp(prefix=f"bkt_{key}_")
        res = run_bass_kernel_spmd(nc, in_maps, core_ids=list(range(NCORE)),
                                   trace=True, tmpdir=td)
        LAST[key] = (res, td)
        return res
    return run_bass_kernel_spmd(nc, in_maps, core_ids=list(range(NCORE)))


def _get_programs():
    if "blocks" not in _CACHE:
        _CACHE["blocks"] = build_blocks()
        _CACHE["head"] = build_head()
    return _CACHE["blocks"], _CACHE["head"]


def _pack_weights(Wr, Wk, Wv, Wo, W1, b1, W2, b2, g1, be1, g2, be2):
    w16 = np.zeros((L, 128, W16), np.float16)
    w32 = np.zeros((L, 128, WS), np.float32)
    for l in range(L):
        his, los = [], []

        def add(mat):  # mat [K, M] fp32 -> hi/lo tiles
            hi, lo = _split16(mat)
            his.append(hi)
            los.append(lo)

        for Wg in (Wr, Wk, Wv):
            for hf in range(KC):
                for kc in range(KC):
                    add(0.5 * Wg[l][kc * 128:(kc + 1) * 128,
                                    hf * 128:(hf + 1) * 128])
        gh = np.concatenate(his, axis=1)
        gl = np.concatenate(los, axis=1)
        his, los = [], []
        for hf in range(KC):
            for kc in range(KC):
                add(Wo[l][kc * 128:(kc + 1) * 128, hf * 128:(hf + 1) * 128])
        woh = np.concatenate(his, axis=1)
        wol = np.concatenate(los, axis=1)
        his, los = [], []
        for mf in range(FC):
            for kc in range(KC):
                add(0.5 * W1[l][kc * 128:(kc + 1) * 128, mf * 128:(mf + 1) * 128])
        w1h = np.concatenate(his, axis=1)
        w1l = np.concatenate(los, axis=1)
        his, los = [], []
        for mh in range(KC):
            for kc8 in range(FC):
                add(0.5 * W2[l][kc8 * 128:(kc8 + 1) * 128,
                                mh * 128:(mh + 1) * 128])
        w2h = np.concatenate(his, axis=1)
        w2l = np.concatenate(los, axis=1)
        w16[l] = np.concatenate([gh, gl, woh, wol, w1h, w1l, w2h, w2l], axis=1)
        w32[l] = np.concatenate([
            0.5 * b1[l].reshape(FC, 128).T,
            0.5 * b2[l].reshape(KC, 128).T,
            g1[l].reshape(KC, 128).T, be1[l].reshape(KC, 128).T,
            g2[l].reshape(KC, 128).T, be2[l].reshape(KC, 128).T,
        ], axis=1)
    return np.ascontiguousarray(w16), np.ascontiguousarray(w32)


def kernel(input_ids, token_embedding, pos_embedding, noise, unif,
           Wr, Wk, Wv, Wo, W1, b1, W2, b2, ln1_g, ln1_b, ln2_g, ln2_b,
           Wout, bout):
    input_ids = np.asarray(input_ids)
    f32 = lambda a: np.asarray(a, dtype=np.float32)
    token_embedding, pos_embedding, noise, unif = map(
        f32, (token_embedding, pos_embedding, noise, unif))
    Wr, Wk, Wv, Wo, W1, b1, W2, b2 = map(f32, (Wr, Wk, Wv, Wo, W1, b1, W2, b2))
    ln1_g, ln1_b, ln2_g, ln2_b, Wout, bout = map(
        f32, (ln1_g, ln1_b, ln2_g, ln2_b, Wout, bout))

    nc_blocks, nc_head = _get_programs()

    spikes = _encode_spikes(input_ids, token_embedding, pos_embedding, noise, unif)
    sp = spikes.reshape(T, NCORE, R, KC, 128)          # (t, core, r, kc, p)
    x0 = np.ascontiguousarray(sp.transpose(1, 4, 3, 0, 2)).reshape(NCORE, 128, KC, TR)
    w16, w32 = _pack_weights(Wr, Wk, Wv, Wo, W1, b1, W2, b2,
                             ln1_g, ln1_b, ln2_g, ln2_b)
    in1 = [{"x0": x0[c], "w16": w16, "w32": w32} for c in range(NCORE)]
    res1 = _run(nc_blocks, in1, "blocks")
    ho = np.stack([res1.results[c]["h_out"].reshape(128, KC, T, R)
                   for c in range(NCORE)])
    hT = np.ascontiguousarray(ho.transpose(1, 2, 3, 0, 4)).reshape(128, KC, TN)
    hTh, hTl = _split16(hT)

    Wp = np.zeros((D, VPAD), np.float32)
    Wp[:, :V] = 0.5 * Wout
    Wph, Wpl = _split16(Wp)
    bp = np.zeros((VPAD,), np.float32)
    bp[:V] = 0.5 * bout
    in2 = []
    for c in range(NCORE):
        def shard(Wx):
            w = Wx[:, c * VSH:(c + 1) * VSH].reshape(KC, 128, VCH, 128)
            return np.ascontiguousarray(
                w.transpose(1, 0, 2, 3)).reshape(128, KC * VCH * 128)
        bsh = np.ascontiguousarray(bp[c * VSH:(c + 1) * VSH].reshape(VCH, 128).T)
        in2.append({"hTh": hTh, "hTl": hTl, "wouth": shard(Wph),
                    "woutl": shard(Wpl), "boutp": bsh})
    res2 = _run(nc_head, in2, "head")
    out_sh = np.stack([res2.results[c]["out_sh"] for c in range(NCORE)])
    out = out_sh.reshape(VPAD, N)[:V]
    out = np.ascontiguousarray(out.T).reshape(B, S, V).astype(np.float32)
    return out
